# revision 7
# baseline (speedup 1.0000x reference)
"""Trainium2 Bass kernel for nn_MemorySystem (cosine-sim attention memory read).

reference:
    x_norm = ||x||_row (B,1); m_norm = ||m||_row (S,1)
    sims = (x @ m^T) / max(x_norm * m_norm^T, 1e-8)
    attn = softmax(8.0 * sims, axis=1)
    out  = attn @ m                       # (B, D)

Sharding: memory_bank rows split across 8 NeuronCores (8192 rows each).
Each core computes, for its shard, the un-normalized softmax numerator
O_c = exp(S_c) @ m_c (B, D) and denominator Z_c = sum_s exp (B,), using the
bounded-score property (|8*cos| <= 8) to skip the max-subtraction entirely.
Per 512-query pass, a ReduceScatter(add) over the bf16 [512, 513] partials
(O|Z) leaves each core with a fully-reduced 64-query slice; it divides O/Z
on-chip and emits those rows. The host reassembles the 8x2 slices.

Optimizations vs the original baseline (355-388us measured); now ~320-330us:
  - mm2 (exp @ m) runs in fp8e4m3 with perf_mode=DoubleRow (2 s-tiles per
    matmul), halving its PE time. Offline simulation against the exact
    seed-0 data puts the end-to-end rel err at 8.8e-3 (gate: 2e-2).
  - pt8 (exp in fp8) is written directly by the ACT Exp instruction; the
    DVE Z-accumulation reads the same fp8 tile, so numerator/denominator
    see identical weights.
  - m/x transposes stay on the PE but are emitted BEHIND the current
    mm1 in the PE FIFO (two-stage load pipeline), so they never starve
    it; m loads are batched 4 tiles per dma_start (issue cost ~0.6us).
  - x prep is single-pass (x stays SBUF-resident; no reload).
  - ReduceScatter payloads are bf16 (half the collective bytes) and the
    post-RS divide chains are emitted AFTER all compute, so the strict
    FIFO DVE queue can never stall the main pipeline on a collective
    (the original lost 25us + a HAM re-throttle to that).
"""

import sys

sys.path.insert(0, "/opt/trn_rl_repo")

import numpy as np
from contextlib import ExitStack

B, S, D = 1024, 65536, 512
NCORES = 8
S_SHARD = S // NCORES  # 8192
P = 128

ST = S_SHARD // P  # 64 s-tiles per core
QT = B // P  # 8 q-tiles
DC = D // P  # 4 d-chunks
# uneven passes: big first pass overlaps the m-load, small last pass keeps
# the final (unoverlapped) ReduceScatter payload small
PASSES = [(0, 512), (512, 512)]  # (q start, q count)
QP = 512  # max pass width (tile allocation size)
QPT = QP // P  # 4 q-tiles max per pass
NPAIR = ST // 2  # 32 s-tile pairs (DoubleRow mm2 granularity)
LAG = 8  # load runs this many s-tiles ahead of pass-0 compute
MB = 4  # m rows DMA'd per batched load (tiles per dma_start)

MAGIC = 0x5F3759DF

_CACHE = {}


def _build():
    import concourse.bass as bass
    import concourse.tile as tile
    from concourse import bacc, mybir
    from concourse.masks import make_identity

    f32 = mybir.dt.float32
    bf16 = mybir.dt.bfloat16
    f8 = mybir.dt.float8e4
    u32 = mybir.dt.uint32
    AF = mybir.ActivationFunctionType
    ALU = mybir.AluOpType
    DR = mybir.MatmulPerfMode.DoubleRow

    nc = bacc.Bacc(None, num_devices=NCORES)
    x_ext = nc.declare_dram_parameter("x", [B, D], f32, isOutput=False)
    m_ext = nc.declare_dram_parameter("mem", [S_SHARD, D], f32, isOutput=False)
    out_ext = nc.declare_dram_parameter("out", [B // NCORES, D], f32, isOutput=True)

    with tile.TileContext(nc) as tc, ExitStack() as ctx:
        persist = ctx.enter_context(tc.tile_pool(name="persist", bufs=1))
        xfp = ctx.enter_context(tc.tile_pool(name="xfp", bufs=1))
        loadp = ctx.enter_context(tc.tile_pool(name="load", bufs=3))
        mbp = ctx.enter_context(tc.tile_pool(name="mbp", bufs=6))
        sqp = ctx.enter_context(tc.tile_pool(name="sqp", bufs=3))
        work = ctx.enter_context(tc.tile_pool(name="work", bufs=2))
        pt8p = ctx.enter_context(tc.tile_pool(name="pt8p", bufs=3))
        zp = ctx.enter_context(tc.tile_pool(name="zp", bufs=2))
        stp = ctx.enter_context(tc.tile_pool(name="stp", bufs=2))
        finp = ctx.enter_context(tc.tile_pool(name="finp", bufs=2))
        dram = ctx.enter_context(tc.tile_pool(name="dram", bufs=4, space="DRAM"))
        # PSUM: 8 banks total. sc(2) + o2(QPT=4) + tp(2) = 8
        psum_sc = ctx.enter_context(tc.tile_pool(name="psc", bufs=2, space="PSUM"))
        psum_o = ctx.enter_context(tc.tile_pool(name="po", bufs=QPT, space="PSUM"))
        psum_tp = ctx.enter_context(tc.tile_pool(name="ptp", bufs=2, space="PSUM"))

        # ---- constants ----
        ident_bf = persist.tile([P, P], bf16)
        make_identity(nc, ident_bf[:])
        ones_f32 = persist.tile([P, 1], f32)
        nc.vector.memset(ones_f32[:], 1.0)
        one_f32 = persist.tile([1, 1], f32)
        nc.vector.memset(one_f32[:], 1.0)
        magic_u = persist.tile([P, 1], u32)
        nc.vector.memset(magic_u[:], MAGIC)

        # ---- persistent SBUF tensors ----
        # [d%128, s//128, d//128, s%128]; per-s-tile [128, 4, 128] blocks are
        # contiguous so the XBAR DMA-transpose can write them directly
        mT = persist.tile([P, ST, DC, P], bf16)
        m8 = persist.tile([P, ST, D], f8)  # [s%128, s//128, d] fp8
        xhatT = persist.tile([P, DC, B], bf16)  # [d%128, d//128, q]
        n2m = persist.tile([P, ST], f32)  # ||m_s||^2 / 64
        rs_m = persist.tile([P, ST], f32)  # 8 / ||m_s||
        rs_u = persist.tile([P, ST], u32)
        rs_t = persist.tile([P, ST], f32)
        xn2 = persist.tile([P, QT], f32)
        rs_x = persist.tile([P, QT], f32)
        xr_u = persist.tile([P, QT], u32)
        xr_t = persist.tile([P, QT], f32)

        def rsqrt_newton(dst, a, uscr, tscr, n):
            """dst = 1/sqrt(a); all APs [P, n] f32 (uscr u32)."""
            mg = magic_u[:, 0:1]
            if n > 1:
                mg = mg.to_broadcast((P, n))
            nc.vector.tensor_scalar(
                uscr, a.bitcast(u32), 1, None, ALU.logical_shift_right
            )
            nc.vector.tensor_tensor(uscr, mg, uscr, ALU.subtract)
            y = uscr.bitcast(f32)
            for it in range(2):
                out_y = dst if it == 1 else y
                nc.vector.tensor_tensor(tscr, y, y, ALU.mult)
                nc.vector.tensor_tensor(tscr, tscr, a, ALU.mult)
                nc.vector.tensor_scalar(tscr, tscr, -0.5, 1.5, ALU.mult, ALU.add)
                nc.vector.tensor_tensor(out_y, y, tscr, ALU.mult)

        # ---- m load pipeline, split in two stages so the PE transposes are
        # emitted AFTER the current iteration's mm1 (they sit behind it in
        # the PE FIFO and never starve it).
        # stage a: batched DMA (MB tiles per dma_start), norms (ACT),
        #          bf16 + fp8 casts (DVE).  stage b: PE transpose + copy-back.
        mf_batches = {}

        def load_stage_a(t):
            b = t // MB
            if b not in mf_batches:
                mfb = loadp.tile([P, MB, D], f32, tag="mf", name=f"mfb_{b}")
                nc.sync.dma_start(
                    out=mfb[:],
                    in_=m_ext[b * MB * P : (b + 1) * MB * P, :].rearrange(
                        "(j p) d -> p j d", p=P
                    ),
                )
                mf_batches[b] = mfb
            mf = mf_batches[b][:, t % MB, :]
            msq = sqp.tile([P, D], f32, tag="sq", name=f"msq_{t}")
            # scale=1/8: accum collects sum((m/8)^2) = n2/64; rsqrt -> 8/||m||
            nc.scalar.activation(
                out=msq[:], in_=mf, func=AF.Square, scale=0.125,
                accum_out=n2m[:, t : t + 1],
            )
            if t % 8 == 7:
                sl = slice(t - 7, t + 1)
                rsqrt_newton(rs_m[:, sl], n2m[:, sl], rs_u[:, sl], rs_t[:, sl], 8)
            mb = mbp.tile([P, D], bf16, tag="mb", name=f"mb_{t}")
            nc.vector.tensor_copy(out=mb[:], in_=mf)
            nc.vector.tensor_copy(out=m8[:, t, :], in_=mb[:])
            return mb

        def load_stage_b(t, mb):
            # XBAR transpose on the DMA engines (no PE/DVE involvement):
            # out[p, c, s'] = mb[s', c*128+p]
            nc.sync.dma_start_transpose(out=mT[:, t], in_=mb[:])

        loaded = set()

        def load_full(t):
            if t < ST and t not in loaded:
                loaded.add(t)
                load_stage_b(t, load_stage_a(t))

        # ---- x prep: two half DMAs.  Half 0 (q 0:512) is prepped before
        # pass-0 starts; half 1 (q 512:1024) is deferred into pass-0's load
        # stream -- pass-0 never reads it, pass-1 starts ~100us later.
        xhalves = []
        for v in range(2):
            xh = xfp.tile([P, QT // 2, D], f32, tag=f"xf{v}", name=f"xfb_{v}")
            nc.sync.dma_start(
                out=xh[:],
                in_=x_ext[v * (B // 2) : (v + 1) * (B // 2), :].rearrange(
                    "(j p) d -> p j d", p=P
                ),
            )
            xhalves.append(xh)

        def xfb(j):
            return xhalves[j // (QT // 2)][:, j % (QT // 2), :]

        def x_square(j):
            xsq = sqp.tile([P, D], f32, tag="sq", name=f"xsq_{j}")
            nc.scalar.activation(
                out=xsq[:], in_=xfb(j), func=AF.Square,
                accum_out=xn2[:, j : j + 1],
            )

        def x_finish(j):
            xhat = work.tile([P, D], bf16, tag="xhat", name=f"xhat_{j}")
            nc.vector.tensor_scalar_mul(xhat[:], xfb(j), rs_x[:, j : j + 1])
            xtp = psum_tp.tile([P, DC * P], bf16, tag="tp", name=f"xtp_{j}")
            for c in range(DC):
                nc.tensor.transpose(
                    xtp[:, c * P : (c + 1) * P],
                    xhat[:, c * P : (c + 1) * P],
                    ident_bf[:],
                )
            nc.vector.tensor_copy(
                out=xhatT[:, :, j * P : (j + 1) * P],
                in_=xtp[:].rearrange("p (c q) -> p c q", c=DC),
            )

        for j in range(QT // 2):
            x_square(j)

        # prime the m pipeline while x-half-0 squares run: DMA/norm/cast for
        # LAG tiles, PE-transpose only tile 0 now; the rest drain into
        # pass-0's loop (2 per iteration) so they sit BEHIND mm1 in the
        # PE FIFO instead of serializing before mm1(0) on the cold clock
        prime_b = []
        for u in range(LAG):
            loaded.add(u)
            mb_u = load_stage_a(u)
            if u == 0:
                load_stage_b(u, mb_u)
            else:
                prime_b.append((u, mb_u))

        rsqrt_newton(rs_x[:, 0 : QT // 2], xn2[:, 0 : QT // 2],
                     xr_u[:, 0 : QT // 2], xr_t[:, 0 : QT // 2], QT // 2)
        for j in range(QT // 2):
            x_finish(j)

        # deferred half-1 thunks, consumed one per pass-0 iteration
        x_thunks = [lambda j=j: x_square(j) for j in range(QT // 2, QT)]
        x_thunks.append(lambda: rsqrt_newton(
            rs_x[:, QT // 2 :], xn2[:, QT // 2 :],
            xr_u[:, QT // 2 :], xr_t[:, QT // 2 :], QT // 2))
        x_thunks.extend([lambda j=j: x_finish(j) for j in range(QT // 2, QT)])

        # ---- main: scores^T -> exp(fp8) -> O (DoubleRow PSUM) / Z (DVE) ----
        rs_jobs = []
        for h, (q0, qp) in enumerate(PASSES):
            qpt = qp // P
            qr = qp // NCORES
            o2 = []
            for j in range(qpt):
                o2.append(psum_o.tile([P, D], f32, tag="o2", name=f"o2_{h}_{j}"))
            zacc = zp.tile([P, 2, QP], f32, tag="zacc", name=f"zacc_{h}")
            nc.gpsimd.memset(zacc[:], 0.0)

            def _mm1(t, q0=q0, qp=qp, h=h):
                sc = psum_sc.tile([P, QP], f32, tag="sc", name=f"sc_{h}_{t}")
                for c in range(DC):
                    nc.tensor.matmul(
                        sc[:, 0:qp],
                        mT[:, t, c, :],
                        xhatT[:, c, q0 : q0 + qp],
                        start=(c == 0),
                        stop=(c == DC - 1),
                    )
                return sc

            def _exp(t, sc, pair, qp=qp):
                nc.scalar.activation(
                    out=pair[:, t % 2, 0:qp], in_=sc[:, 0:qp], func=AF.Exp,
                    scale=rs_m[:, t : t + 1],
                )

            def _mm2(u, pair, o2=o2, qpt=qpt):
                for j in range(qpt):
                    nc.tensor.matmul(
                        o2[j][:],
                        pair[:, :, j * P : (j + 1) * P],
                        m8[:, 2 * u : 2 * u + 2, :],
                        start=(u == 0),
                        stop=(u == NPAIR - 1),
                        perf_mode=DR,
                    )

            def _zadd(u, pair, zacc=zacc, qp=qp):
                # one GpSimd op accumulates the whole [P, 2, qp] pair (GpSimd
                # is otherwise idle; DVE is co-bottleneck in pass 0); the two
                # halves (even/odd s-tile) are summed later by the two
                # accumulating zsum matmuls
                nc.gpsimd.tensor_tensor(
                    zacc[:, :, 0:qp],
                    zacc[:, :, 0:qp],
                    pair[:, :, 0:qp],
                    ALU.add,
                )

            pairs = {}
            pending_b = list(prime_b) if h == 0 else []
            prime_b = []
            for t in range(ST):
                if h == 0:
                    ta = t + LAG
                    if ta < ST and ta not in loaded:
                        loaded.add(ta)
                        mb = load_stage_a(ta)
                        pending_b.append((ta, mb))
                sc = _mm1(t)
                u = t // 2
                if t % 2 == 0:
                    pairs[u] = pt8p.tile(
                        [P, 2, QP], f8, tag="pt8", name=f"pt8_{h}_{u}"
                    )
                _exp(t, sc, pairs[u])
                if h == 0 and pending_b:
                    load_stage_b(*pending_b.pop(0))
                if h == 0 and len(pending_b) > 1:
                    load_stage_b(*pending_b.pop(0))
                if h == 0 and x_thunks:
                    x_thunks.pop(0)()
                if t % 2 == 1:
                    _zadd(u, pairs[u])
                if t >= 2 and t % 2 == 0:
                    _mm2(u - 1, pairs.pop(u - 1))
            _mm2(NPAIR - 1, pairs.pop(NPAIR - 1))

            # cross-partition Z reduce: [1, qp] = ones^T @ (zacc[0] + zacc[1])
            zsum = psum_tp.tile([1, QP], f32, tag="tp", name=f"zsum_{h}")
            nc.tensor.matmul(
                zsum[:, 0:qp], ones_f32[:], zacc[:, 0, 0:qp], start=True, stop=False
            )
            nc.tensor.matmul(
                zsum[:, 0:qp], ones_f32[:], zacc[:, 1, 0:qp], start=False, stop=True
            )
            zrow = finp.tile([1, QP], f32, tag="zrow", name=f"zrow_{h}")
            nc.vector.tensor_copy(out=zrow[0:1, 0:qp], in_=zsum[0:1, 0:qp])
            ztp = psum_tp.tile([P, QPT], f32, tag="tp", name=f"ztp_{h}")
            for j in range(qpt):
                nc.tensor.transpose(
                    ztp[:, j : j + 1], zrow[0:1, j * P : (j + 1) * P], one_f32[:]
                )

            # stage [128, qpt, D+1] bf16: cols 0..D-1 = O, col D = Z
            stage = stp.tile([P, QPT, D + 1], bf16, tag="stage", name=f"stage_{h}")
            for j in range(qpt):
                nc.vector.tensor_copy(out=stage[:, j, 0:D], in_=o2[j][:])
            nc.vector.tensor_copy(
                out=stage[:, 0:qpt, D : D + 1],
                in_=ztp[:, 0:qpt].rearrange("p (j o) -> p j o", o=1),
            )
            partial = dram.tile(
                [qp, D + 1], bf16, tag=f"partial{qp}", name=f"partial_{h}",
                bufs=2,
            )
            nc.sync.dma_start(
                out=partial[:].rearrange("(o p) d -> p o d", p=P),
                in_=stage[:, 0:qpt, :],
            )
            a2a = dram.tile(
                [qp, D + 1], bf16, tag=f"a2a{qp}", name=f"a2a_{h}", bufs=2
            )
            nc.gpsimd.collective_compute(
                "AllToAll",
                mybir.AluOpType.bypass,
                replica_groups=[list(range(NCORES))],
                ins=[partial[:].opt()],
                outs=[a2a[:].opt()],
            )
            rs_jobs.append((h, q0, qp, a2a))

        # post-RS divide + output: pushed to the end of the scheduler's
        # simulated timeline (tile_wait_until) so the strict-FIFO engine
        # queues can never stall mid-kernel on a collective dependency
        for h, q0, qp, a2a in rs_jobs:
            qr = qp // NCORES
            ctx2 = tc.tile_wait_until(5.0)
            ctx2.__enter__()
            # a2a rows [64c : 64c+64] = core c's partial for MY query slice;
            # sum the 8 blocks locally (the reduce the RS used to do)
            fin8 = finp.tile(
                [qr, NCORES, D + 1], bf16, tag=f"fin8{qp}", name=f"fin8_{h}"
            )
            nc.sync.dma_start(
                out=fin8[:], in_=a2a[:].rearrange("(c p) d -> p c d", p=qr)
            )
            acc = finp.tile([qr, D + 1], f32, tag=f"acc{qp}", name=f"acc_{h}")
            nc.vector.tensor_tensor(
                acc[:], fin8[:, 0, :], fin8[:, 1, :], ALU.add
            )
            for c in range(2, NCORES):
                nc.vector.tensor_tensor(acc[:], acc[:], fin8[:, c, :], ALU.add)
            rz = finp.tile([qr, 1], f32, tag=f"rz{qp}", name=f"rz_{h}")
            nc.vector.reciprocal(rz[:], acc[:, D : D + 1])
            outb = finp.tile([qr, D], f32, tag=f"outb{qp}", name=f"outb_{h}")
            nc.vector.tensor_scalar_mul(outb[:], acc[:, 0:D], rz[:])
            nc.sync.dma_start(
                out=out_ext[q0 // NCORES : q0 // NCORES + qr, :], in_=outb[:]
            )
            ctx2.__exit__(None, None, None)

    nc.compile()
    return nc


def _get_nc():
    if "nc" not in _CACHE:
        _CACHE["nc"] = _build()
    return _CACHE["nc"]


def _run(x, memory_bank, trace=False, **trace_kwargs):
    from concourse.bass_utils import run_bass_kernel_spmd

    nc = _get_nc()
    x = np.ascontiguousarray(np.asarray(x, dtype=np.float32))
    memory_bank = np.ascontiguousarray(np.asarray(memory_bank, dtype=np.float32))
    in_maps = [
        {
            "x": x,
            "mem": np.ascontiguousarray(
                memory_bank[i * S_SHARD : (i + 1) * S_SHARD]
            ),
        }
        for i in range(NCORES)
    ]
    res = run_bass_kernel_spmd(
        nc, in_maps, list(range(NCORES)), trace=trace, **trace_kwargs
    )
    # core i's output rows q0/8..q0/8+qr hold global q rows q0 + i*qr + k
    out = np.empty((B, D), dtype=np.float32)
    for i in range(NCORES):
        r = np.asarray(res.results[i]["out"])
        for q0, qp in PASSES:
            qr = qp // NCORES
            out[q0 + i * qr : q0 + (i + 1) * qr] = r[
                q0 // NCORES : q0 // NCORES + qr
            ]
    return out, res


def kernel(x, memory_bank):
    out, _ = _run(x, memory_bank)
    return out


if __name__ == "__main__":
    xs = np.random.randn(B, D).astype(np.float32)
    ms = np.random.randn(S, D).astype(np.float32)
    o = kernel(xs, ms)
    print(o.shape, o.dtype)



# revision 15
# speedup vs baseline: 1.1537x; 1.1537x over previous
"""Trainium2 Bass kernel for nn_MemorySystem (cosine-sim attention memory read).

reference:
    x_norm = ||x||_row (B,1); m_norm = ||m||_row (S,1)
    sims = (x @ m^T) / max(x_norm * m_norm^T, 1e-8)
    attn = softmax(8.0 * sims, axis=1)
    out  = attn @ m                       # (B, D)

Sharding: memory_bank rows split across 8 NeuronCores (8192 rows each).
Each core computes, for its shard, the un-normalized softmax numerator
O_c = exp(S_c) @ m_c (B, D) and denominator Z_c = sum_s exp (B,), using the
bounded-score property (|8*cos| <= 8) to skip the max-subtraction entirely.
Per 512-query pass, a ReduceScatter(add) over the bf16 [512, 513] partials
(O|Z) leaves each core with a fully-reduced 64-query slice; it divides O/Z
on-chip and emits those rows. The host reassembles the 8x2 slices.

Optimizations vs the original baseline (355-388us measured); now ~320-330us:
  - mm2 (exp @ m) runs in fp8e4m3 with perf_mode=DoubleRow (2 s-tiles per
    matmul), halving its PE time. Offline simulation against the exact
    seed-0 data puts the end-to-end rel err at 8.8e-3 (gate: 2e-2).
  - pt8 (exp in fp8) is written directly by the ACT Exp instruction; the
    DVE Z-accumulation reads the same fp8 tile, so numerator/denominator
    see identical weights.
  - m/x transposes stay on the PE but are emitted BEHIND the current
    mm1 in the PE FIFO (two-stage load pipeline), so they never starve
    it; m loads are batched 4 tiles per dma_start (issue cost ~0.6us).
  - x prep is single-pass (x stays SBUF-resident; no reload).
  - ReduceScatter payloads are bf16 (half the collective bytes) and the
    post-RS divide chains are emitted AFTER all compute, so the strict
    FIFO DVE queue can never stall the main pipeline on a collective
    (the original lost 25us + a HAM re-throttle to that).
"""

import sys

sys.path.insert(0, "/opt/trn_rl_repo")

import numpy as np
from contextlib import ExitStack

B, S, D = 1024, 65536, 512
NCORES = 8
S_SHARD = S // NCORES  # 8192
P = 128

ST = S_SHARD // P  # 64 s-tiles per core
QT = B // P  # 8 q-tiles
DC = D // P  # 4 d-chunks
# uneven passes: big first pass overlaps the m-load, small last pass keeps
# the final (unoverlapped) ReduceScatter payload small
PASSES = [(0, 512), (512, 512)]  # (q start, q count)
# collective chunks: pass 1's A2A is split [384, 128] to shrink the tail
CHUNKS = [(0, 512), (512, 384), (896, 128)]
QP = 512  # max pass width (tile allocation size)
QPT = QP // P  # 4 q-tiles max per pass
NPAIR = ST // 2  # 32 s-tile pairs (DoubleRow mm2 granularity)
LAG = 8  # load runs this many s-tiles ahead of pass-0 compute
MB = 4  # m rows DMA'd per batched load (tiles per dma_start)

MAGIC = 0x5F3759DF

_CACHE = {}


def _build():
    import concourse.bass as bass
    import concourse.tile as tile
    from concourse import bacc, mybir
    from concourse.masks import make_identity

    f32 = mybir.dt.float32
    bf16 = mybir.dt.bfloat16
    f8 = mybir.dt.float8e4
    u32 = mybir.dt.uint32
    AF = mybir.ActivationFunctionType
    ALU = mybir.AluOpType
    DR = mybir.MatmulPerfMode.DoubleRow

    nc = bacc.Bacc(None, num_devices=NCORES)
    x_ext = nc.declare_dram_parameter("x", [B, D], f32, isOutput=False)
    m_ext = nc.declare_dram_parameter("mem", [S_SHARD, D], f32, isOutput=False)
    out_ext = nc.declare_dram_parameter("out", [B // NCORES, D], f32, isOutput=True)

    with tile.TileContext(nc) as tc, ExitStack() as ctx:
        persist = ctx.enter_context(tc.tile_pool(name="persist", bufs=1))
        xfp = ctx.enter_context(tc.tile_pool(name="xfp", bufs=1))
        loadp = ctx.enter_context(tc.tile_pool(name="load", bufs=3))
        mbp = ctx.enter_context(tc.tile_pool(name="mbp", bufs=6))
        sqp = ctx.enter_context(tc.tile_pool(name="sqp", bufs=3))
        work = ctx.enter_context(tc.tile_pool(name="work", bufs=2))
        pt8p = ctx.enter_context(tc.tile_pool(name="pt8p", bufs=3))
        zp = ctx.enter_context(tc.tile_pool(name="zp", bufs=2))
        stp = ctx.enter_context(tc.tile_pool(name="stp", bufs=2))
        finp = ctx.enter_context(tc.tile_pool(name="finp", bufs=2))
        dram = ctx.enter_context(tc.tile_pool(name="dram", bufs=4, space="DRAM"))
        # PSUM: 8 banks total. sc(2) + o2(QPT=4) + tp(2) = 8
        psum_sc = ctx.enter_context(tc.tile_pool(name="psc", bufs=2, space="PSUM"))
        psum_o = ctx.enter_context(tc.tile_pool(name="po", bufs=QPT, space="PSUM"))
        psum_tp = ctx.enter_context(tc.tile_pool(name="ptp", bufs=2, space="PSUM"))

        # ---- constants ----
        ident_bf = persist.tile([P, P], bf16)
        make_identity(nc, ident_bf[:])
        ones_f32 = persist.tile([P, 1], f32)
        nc.vector.memset(ones_f32[:], 1.0)
        one_f32 = persist.tile([1, 1], f32)
        nc.vector.memset(one_f32[:], 1.0)
        magic_u = persist.tile([P, 1], u32)
        nc.vector.memset(magic_u[:], MAGIC)

        # ---- persistent SBUF tensors ----
        mT = persist.tile([P, DC, S_SHARD], bf16)  # [d%128, d//128, s]
        m8 = persist.tile([P, ST, D], f8)  # [s%128, s//128, d] fp8
        xhatT = persist.tile([P, DC, B], bf16)  # [d%128, d//128, q]
        n2m = persist.tile([P, ST], f32)  # ||m_s||^2 / 64
        rs_m = persist.tile([P, ST], f32)  # 8 / ||m_s||
        rs_u = persist.tile([P, ST], u32)
        rs_t = persist.tile([P, ST], f32)
        xn2 = persist.tile([P, QT], f32)
        rs_x = persist.tile([P, QT], f32)
        xr_u = persist.tile([P, QT], u32)
        xr_t = persist.tile([P, QT], f32)

        def rsqrt_newton(dst, a, uscr, tscr, n):
            """dst = 1/sqrt(a); all APs [P, n] f32 (uscr u32)."""
            mg = magic_u[:, 0:1]
            if n > 1:
                mg = mg.to_broadcast((P, n))
            nc.vector.tensor_scalar(
                uscr, a.bitcast(u32), 1, None, ALU.logical_shift_right
            )
            nc.vector.tensor_tensor(uscr, mg, uscr, ALU.subtract)
            y = uscr.bitcast(f32)
            for it in range(2):
                out_y = dst if it == 1 else y
                nc.vector.tensor_tensor(tscr, y, y, ALU.mult)
                nc.vector.tensor_tensor(tscr, tscr, a, ALU.mult)
                nc.vector.tensor_scalar(tscr, tscr, -0.5, 1.5, ALU.mult, ALU.add)
                nc.vector.tensor_tensor(out_y, y, tscr, ALU.mult)

        # ---- m load pipeline, split in two stages so the PE transposes are
        # emitted AFTER the current iteration's mm1 (they sit behind it in
        # the PE FIFO and never starve it).
        # stage a: batched DMA (MB tiles per dma_start), norms (ACT),
        #          bf16 + fp8 casts (DVE).  stage b: PE transpose + copy-back.
        mf_batches = {}

        def load_stage_a(t):
            b = t // MB
            if b not in mf_batches:
                mfb = loadp.tile([P, MB, D], f32, tag="mf", name=f"mfb_{b}")
                nc.sync.dma_start(
                    out=mfb[:],
                    in_=m_ext[b * MB * P : (b + 1) * MB * P, :].rearrange(
                        "(j p) d -> p j d", p=P
                    ),
                )
                mf_batches[b] = mfb
            mf = mf_batches[b][:, t % MB, :]
            msq = sqp.tile([P, D], f32, tag="sq", name=f"msq_{t}")
            # scale=1/8: accum collects sum((m/8)^2) = n2/64; rsqrt -> 8/||m||
            nc.scalar.activation(
                out=msq[:], in_=mf, func=AF.Square, scale=0.125,
                accum_out=n2m[:, t : t + 1],
            )
            if t % 8 == 7:
                sl = slice(t - 7, t + 1)
                rsqrt_newton(rs_m[:, sl], n2m[:, sl], rs_u[:, sl], rs_t[:, sl], 8)
            mb = mbp.tile([P, D], bf16, tag="mb", name=f"mb_{t}")
            nc.vector.tensor_copy(out=mb[:], in_=mf)
            nc.vector.tensor_copy(out=m8[:, t, :], in_=mb[:])
            return mb

        def load_stage_b(t, mb):
            mtp = psum_tp.tile([P, DC * P], bf16, tag="tp", name=f"mtp_{t}")
            for c in range(DC):
                nc.tensor.transpose(
                    mtp[:, c * P : (c + 1) * P],
                    mb[:, c * P : (c + 1) * P],
                    ident_bf[:],
                )
            nc.vector.tensor_copy(
                out=mT[:, :, t * P : (t + 1) * P],
                in_=mtp[:].rearrange("p (c q) -> p c q", c=DC),
            )

        loaded = set()

        def load_full(t):
            if t < ST and t not in loaded:
                loaded.add(t)
                load_stage_b(t, load_stage_a(t))

        # ---- x prep: two half DMAs.  Half 0 (q 0:512) is prepped before
        # pass-0 starts; half 1 (q 512:1024) is deferred into pass-0's load
        # stream -- pass-0 never reads it, pass-1 starts ~100us later.
        xhalves = []
        for v in range(2):
            xh = xfp.tile([P, QT // 2, D], f32, tag=f"xf{v}", name=f"xfb_{v}")
            nc.sync.dma_start(
                out=xh[:],
                in_=x_ext[v * (B // 2) : (v + 1) * (B // 2), :].rearrange(
                    "(j p) d -> p j d", p=P
                ),
            )
            xhalves.append(xh)

        def xfb(j):
            return xhalves[j // (QT // 2)][:, j % (QT // 2), :]

        def x_square(j):
            xsq = sqp.tile([P, D], f32, tag="sq", name=f"xsq_{j}")
            nc.scalar.activation(
                out=xsq[:], in_=xfb(j), func=AF.Square,
                accum_out=xn2[:, j : j + 1],
            )

        def x_finish(j):
            xhat = work.tile([P, D], bf16, tag="xhat", name=f"xhat_{j}")
            nc.vector.tensor_scalar_mul(xhat[:], xfb(j), rs_x[:, j : j + 1])
            xtp = psum_tp.tile([P, DC * P], bf16, tag="tp", name=f"xtp_{j}")
            for c in range(DC):
                nc.tensor.transpose(
                    xtp[:, c * P : (c + 1) * P],
                    xhat[:, c * P : (c + 1) * P],
                    ident_bf[:],
                )
            nc.vector.tensor_copy(
                out=xhatT[:, :, j * P : (j + 1) * P],
                in_=xtp[:].rearrange("p (c q) -> p c q", c=DC),
            )

        for j in range(QT // 2):
            x_square(j)

        # prime the m pipeline while x-half-0 squares run: DMA/norm/cast for
        # LAG tiles, PE-transpose only tile 0 now; the rest drain into
        # pass-0's loop (2 per iteration) so they sit BEHIND mm1 in the
        # PE FIFO instead of serializing before mm1(0) on the cold clock
        prime_b = []
        for u in range(LAG):
            loaded.add(u)
            mb_u = load_stage_a(u)
            if u == 0:
                load_stage_b(u, mb_u)
            else:
                prime_b.append((u, mb_u))

        rsqrt_newton(rs_x[:, 0 : QT // 2], xn2[:, 0 : QT // 2],
                     xr_u[:, 0 : QT // 2], xr_t[:, 0 : QT // 2], QT // 2)
        for j in range(QT // 2):
            x_finish(j)

        # deferred half-1 thunks, consumed one per pass-0 iteration
        x_thunks = [lambda j=j: x_square(j) for j in range(QT // 2, QT)]
        x_thunks.append(lambda: rsqrt_newton(
            rs_x[:, QT // 2 :], xn2[:, QT // 2 :],
            xr_u[:, QT // 2 :], xr_t[:, QT // 2 :], QT // 2))
        x_thunks.extend([lambda j=j: x_finish(j) for j in range(QT // 2, QT)])

        # ---- main: scores^T -> exp(fp8) -> O (DoubleRow PSUM) / Z (DVE) ----
        rs_jobs = []
        for h, (q0, qp) in enumerate(PASSES):
            qpt = qp // P
            qr = qp // NCORES
            o2 = []
            for j in range(qpt):
                o2.append(psum_o.tile([P, D], f32, tag="o2", name=f"o2_{h}_{j}"))
            zacc = zp.tile([P, 2, QP], f32, tag="zacc", name=f"zacc_{h}")
            nc.gpsimd.memset(zacc[:], 0.0)

            def _mm1(t, q0=q0, qp=qp, h=h):
                sc = psum_sc.tile([P, QP], f32, tag="sc", name=f"sc_{h}_{t}")
                for c in range(DC):
                    nc.tensor.matmul(
                        sc[:, 0:qp],
                        mT[:, c, t * P : (t + 1) * P],
                        xhatT[:, c, q0 : q0 + qp],
                        start=(c == 0),
                        stop=(c == DC - 1),
                    )
                return sc

            def _exp(t, sc, pair, qp=qp):
                nc.scalar.activation(
                    out=pair[:, t % 2, 0:qp], in_=sc[:, 0:qp], func=AF.Exp,
                    scale=rs_m[:, t : t + 1],
                )

            def _mm2(u, pair, o2=o2, qpt=qpt):
                for j in range(qpt):
                    nc.tensor.matmul(
                        o2[j][:],
                        pair[:, :, j * P : (j + 1) * P],
                        m8[:, 2 * u : 2 * u + 2, :],
                        start=(u == 0),
                        stop=(u == NPAIR - 1),
                        perf_mode=DR,
                    )

            def _zadd(u, pair, zacc=zacc, qp=qp):
                # one GpSimd op accumulates the whole [P, 2, qp] pair (GpSimd
                # is otherwise idle; DVE is co-bottleneck in pass 0); the two
                # halves (even/odd s-tile) are summed later by the two
                # accumulating zsum matmuls
                nc.gpsimd.tensor_tensor(
                    zacc[:, :, 0:qp],
                    zacc[:, :, 0:qp],
                    pair[:, :, 0:qp],
                    ALU.add,
                )

            pairs = {}
            pending_b = list(prime_b) if h == 0 else []
            prime_b = []
            for t in range(ST):
                if h == 0:
                    ta = t + LAG
                    if ta < ST and ta not in loaded:
                        loaded.add(ta)
                        mb = load_stage_a(ta)
                        pending_b.append((ta, mb))
                sc = _mm1(t)
                u = t // 2
                if t % 2 == 0:
                    pairs[u] = pt8p.tile(
                        [P, 2, QP], f8, tag="pt8", name=f"pt8_{h}_{u}"
                    )
                _exp(t, sc, pairs[u])
                if h == 0 and pending_b:
                    load_stage_b(*pending_b.pop(0))
                if h == 0 and len(pending_b) > 1:
                    load_stage_b(*pending_b.pop(0))
                if h == 0 and x_thunks:
                    x_thunks.pop(0)()
                if t % 2 == 1:
                    _zadd(u, pairs[u])
                if t >= 2 and t % 2 == 0:
                    _mm2(u - 1, pairs.pop(u - 1))
            _mm2(NPAIR - 1, pairs.pop(NPAIR - 1))

            # cross-partition Z reduce: [1, qp] = ones^T @ (zacc[0] + zacc[1])
            zsum = psum_tp.tile([1, QP], f32, tag="tp", name=f"zsum_{h}")
            nc.tensor.matmul(
                zsum[:, 0:qp], ones_f32[:], zacc[:, 0, 0:qp], start=True, stop=False
            )
            nc.tensor.matmul(
                zsum[:, 0:qp], ones_f32[:], zacc[:, 1, 0:qp], start=False, stop=True
            )
            zrow = finp.tile([1, QP], f32, tag="zrow", name=f"zrow_{h}")
            nc.vector.tensor_copy(out=zrow[0:1, 0:qp], in_=zsum[0:1, 0:qp])
            ztp = psum_tp.tile([P, QPT], f32, tag="tp", name=f"ztp_{h}")
            for j in range(qpt):
                nc.tensor.transpose(
                    ztp[:, j : j + 1], zrow[0:1, j * P : (j + 1) * P], one_f32[:]
                )

            # stage [128, qpt, D+1] bf16: cols 0..D-1 = O, col D = Z
            stage = stp.tile([P, QPT, D + 1], bf16, tag="stage", name=f"stage_{h}")
            for j in range(qpt):
                nc.vector.tensor_copy(out=stage[:, j, 0:D], in_=o2[j][:])
            nc.vector.tensor_copy(
                out=stage[:, 0:qpt, D : D + 1],
                in_=ztp[:, 0:qpt].rearrange("p (j o) -> p j o", o=1),
            )
            # the LAST pass splits its A2A into [384, 128] chunks: only the
            # small trailing chunk's collective is exposed; each chunk's
            # post-chain overlaps the next chunk's A2A
            if h == len(PASSES) - 1:
                chunks = [(q0, 384), (q0 + 384, 128)]
            else:
                chunks = [(q0, qp)]
            ro = 0
            for cq0, cqp in chunks:
                cjt = cqp // P
                partial = dram.tile(
                    [cqp, D + 1], bf16, tag=f"partial{cqp}",
                    name=f"partial_{h}_{cq0}", bufs=1,
                )
                nc.sync.dma_start(
                    out=partial[:].rearrange("(o p) d -> p o d", p=P),
                    in_=stage[:, ro : ro + cjt, :],
                )
                a2a = dram.tile(
                    [cqp, D + 1], bf16, tag=f"a2a{cqp}",
                    name=f"a2a_{h}_{cq0}", bufs=1,
                )
                nc.gpsimd.collective_compute(
                    "AllToAll",
                    mybir.AluOpType.bypass,
                    replica_groups=[list(range(NCORES))],
                    ins=[partial[:].opt()],
                    outs=[a2a[:].opt()],
                )
                rs_jobs.append((h, cq0, cqp, a2a))
                ro += cjt

        # post-RS divide + output: pushed to the end of the scheduler's
        # simulated timeline (tile_wait_until) so the strict-FIFO engine
        # queues can never stall mid-kernel on a collective dependency
        for h, q0, qp, a2a in rs_jobs:
            qr = qp // NCORES
            ctx2 = tc.tile_wait_until(5.0)
            ctx2.__enter__()
            # a2a rows [qr*c : qr*(c+1)] = core c's partial for MY query
            # slice; sum the 8 blocks locally (the reduce the RS used to do)
            fin8 = finp.tile(
                [QP // NCORES, NCORES, D + 1], bf16, tag="fin8",
                name=f"fin8_{q0}",
            )[0:qr]
            nc.sync.dma_start(
                out=fin8[:], in_=a2a[:].rearrange("(c p) d -> p c d", p=qr)
            )
            acc = finp.tile(
                [QP // NCORES, D + 1], f32, tag="acc", name=f"acc_{q0}"
            )[0:qr]
            nc.vector.tensor_tensor(
                acc[:], fin8[:, 0, :], fin8[:, 1, :], ALU.add
            )
            for c in range(2, NCORES):
                nc.vector.tensor_tensor(acc[:], acc[:], fin8[:, c, :], ALU.add)
            rz = finp.tile(
                [QP // NCORES, 1], f32, tag="rz", name=f"rz_{q0}"
            )[0:qr]
            nc.vector.reciprocal(rz[:], acc[:, D : D + 1])
            outb = finp.tile(
                [QP // NCORES, D], f32, tag="outb", name=f"outb_{q0}"
            )[0:qr]
            nc.vector.tensor_scalar_mul(outb[:], acc[:, 0:D], rz[:])
            nc.sync.dma_start(
                out=out_ext[q0 // NCORES : q0 // NCORES + qr, :], in_=outb[:]
            )
            ctx2.__exit__(None, None, None)

    nc.compile()
    return nc


def _get_nc():
    if "nc" not in _CACHE:
        _CACHE["nc"] = _build()
    return _CACHE["nc"]


def _run(x, memory_bank, trace=False, **trace_kwargs):
    from concourse.bass_utils import run_bass_kernel_spmd

    nc = _get_nc()
    x = np.ascontiguousarray(np.asarray(x, dtype=np.float32))
    memory_bank = np.ascontiguousarray(np.asarray(memory_bank, dtype=np.float32))
    in_maps = [
        {
            "x": x,
            "mem": np.ascontiguousarray(
                memory_bank[i * S_SHARD : (i + 1) * S_SHARD]
            ),
        }
        for i in range(NCORES)
    ]
    res = run_bass_kernel_spmd(
        nc, in_maps, list(range(NCORES)), trace=trace, **trace_kwargs
    )
    # core i's output rows q0/8..q0/8+qr hold global q rows q0 + i*qr + k
    out = np.empty((B, D), dtype=np.float32)
    for i in range(NCORES):
        r = np.asarray(res.results[i]["out"])
        for q0, qp in CHUNKS:
            qr = qp // NCORES
            out[q0 + i * qr : q0 + (i + 1) * qr] = r[
                q0 // NCORES : q0 // NCORES + qr
            ]
    return out, res


def kernel(x, memory_bank):
    out, _ = _run(x, memory_bank)
    return out


if __name__ == "__main__":
    xs = np.random.randn(B, D).astype(np.float32)
    ms = np.random.randn(S, D).astype(np.float32)
    o = kernel(xs, ms)
    print(o.shape, o.dtype)

